# revision 9
# baseline (speedup 1.0000x reference)
"""Child-Sum TreeLSTM over a complete 4-ary forest — Trainium2 Bass kernel.

Layout: "transposed space" — memory dim (150) on SBUF partitions (split
128+22), nodes on the free dim.  Each of the 8 cores owns a contiguous 1/8
shard of every level 0..6; children of a core's parents at level d are
exactly the core's shard of level d-1, so levels 0..6 need no cross-core
communication.  Levels 7 (4 nodes) + 8 (1 node) are finished on the host
from each core's exported level-6 h/c (5 of 87381 nodes).

Matmul operands (x, weights, child-h) are bf16; PSUM accumulation, gates,
biases, c-state and all outputs stay fp32.

Per-core device inputs:
  xT   (300, 10922) bf16  embs^T, level blocks [L0 8192|L1 2048|...|L6 2]
  wx   (300, 600)   bf16  [W_ix | W_ux | W_ox | W_fx]
  wh   (150, 600)   bf16  [W_ih | W_uh | W_oh | W_fh]
  bias (150, 4)     fp32  combined [b_ix+b_ih, b_ux+b_uh, b_ox+b_oh, b_fx+b_fh]
Outputs (fp32):
  hT   (150, 10922)  h for the core's rows of levels 0..6 (transposed)
  hc6  (150, 4)      [h6 n0, h6 n1, c6 n0, c6 n1] for host top-levels
"""

import sys
import numpy as np
import ml_dtypes

for p in ("/opt/trn_rl_repo",):
    if p not in sys.path:
        sys.path.append(p)

import concourse.bass as bass
import concourse.bacc as bacc
import concourse.tile as tile
from concourse import mybir
from concourse.bass_utils import run_bass_kernel_spmd

F32 = mybir.dt.float32
BF16 = mybir.dt.bfloat16
LAST_EXEC_NS = None
LAST_IN_MAPS = None
AF = mybir.ActivationFunctionType
ALU = mybir.AluOpType

IN_DIM, MEM, K, D = 300, 150, 4, 9
SIZES = [K ** (D - 1 - d) for d in range(D)]          # [65536, ..., 1]
N = sum(SIZES)                                        # 87381
NCORES = 8
S = [SIZES[d] // NCORES for d in range(7)]            # [8192,2048,512,128,32,8,2]
NC_COLS = sum(S)                                      # 10922
OFF = [0]
for d in range(7):
    OFF.append(OFF[-1] + S[d])
GOFF = [0]
for d in range(D):
    GOFF.append(GOFF[-1] + SIZES[d])

KC_X = [(0, 128), (128, 256), (256, 300)]             # K chunks of IN_DIM
KC_H = [(0, 128), (128, 150)]                         # K chunks of MEM
MC = [(0, 128), (128, 150)]                           # M chunks of MEM
GATE_I, GATE_U, GATE_O, GATE_F = 0, 1, 2, 3
GFUNC = {GATE_I: AF.Sigmoid, GATE_U: AF.Tanh, GATE_O: AF.Sigmoid}


def _build_program():
    nc = bacc.Bacc()
    xT = nc.declare_dram_parameter("xT", [IN_DIM, NC_COLS], BF16, isOutput=False)
    wx = nc.declare_dram_parameter("wx", [IN_DIM, 600], BF16, isOutput=False)
    wh = nc.declare_dram_parameter("wh", [MEM, 600], BF16, isOutput=False)
    bias = nc.declare_dram_parameter("bias", [MEM, 4], F32, isOutput=False)
    hT = nc.declare_dram_parameter("hT", [MEM, NC_COLS], F32, isOutput=True)
    hc6 = nc.declare_dram_parameter("hc6", [MEM, 4], F32, isOutput=True)

    with tile.TileContext(nc) as tc:
        with (
            tc.tile_pool(name="consts", bufs=1) as consts,
            tc.tile_pool(name="xs", bufs=3) as xs,
            tc.tile_pool(name="gates", bufs=2) as gates,
            tc.tile_pool(name="fwide", bufs=1) as fwide,
            tc.tile_pool(name="leafst", bufs=1) as leafst,
            tc.tile_pool(name="state", bufs=1) as state,
            tc.tile_pool(name="pm", bufs=4, space="PSUM") as pmpool,
            tc.tile_pool(name="pr", bufs=4, space="PSUM") as prpool,
        ):
            # ---- load weights / biases once ----
            wx_t, wh_t = [], []
            for i, (a, b) in enumerate(KC_X):
                t = consts.tile([b - a, 600], BF16, tag=f"wx{i}", name=f"wx{i}")
                nc.gpsimd.dma_start(out=t, in_=wx[a:b, :])
                wx_t.append(t)
            for i, (a, b) in enumerate(KC_H):
                t = consts.tile([b - a, 600], BF16, tag=f"wh{i}", name=f"wh{i}")
                nc.gpsimd.dma_start(out=t, in_=wh[a:b, :])
                wh_t.append(t)
            bias_t = []
            for i, (a, b) in enumerate(MC):
                t = consts.tile([b - a, 4], F32, tag=f"b{i}", name=f"b{i}")
                nc.gpsimd.dma_start(out=t, in_=bias[a:b, :])
                bias_t.append(t)
            # ACT-engine warmup touch of each bias tile: absorbs the bias-DMA
            # wait into a tiny 1-wait instruction so later psum-evac ACTs only
            # wait on PE (walrus allows very few sync commands per instr).
            for i in range(2):
                wu = consts.tile([MC[i][1] - MC[i][0], 1], F32,
                                 tag=f"wu{i}", name=f"wu{i}")
                nc.scalar.copy(out=wu, in_=bias_t[i][:, 0:1])

            def load_x(col, w):
                ts = []
                for i, (a, b) in enumerate(KC_X):
                    t = xs.tile([b - a, w], BF16, tag=f"x{i}", name=f"x{i}")
                    nc.gpsimd.dma_start(out=t, in_=xT[a:b, col:col + w])
                    ts.append(t)
                return ts

            def new_psum(mc, w):
                if mc == 0:
                    return pmpool.tile([128, w], F32, tag="pm", name="pm")
                return prpool.tile([22, w], F32, tag="pr", name="pr")

            def xgate_mms(ps, x_t, g, mc, w, start=True, stop=True):
                m0, m1 = MC[mc]
                for kc in range(3):
                    nc.tensor.matmul(
                        out=ps[:, :w],
                        lhsT=wx_t[kc][:, g * 150 + m0: g * 150 + m1],
                        rhs=x_t[kc][:, :w],
                        start=(start and kc == 0),
                        stop=(stop and kc == 2),
                    )

            def hgate_mms(ps, hs_pair, g, mc, w, start, stop):
                m0, m1 = MC[mc]
                for kc in range(2):
                    nc.tensor.matmul(
                        out=ps[:, :w],
                        lhsT=wh_t[kc][:, g * 150 + m0: g * 150 + m1],
                        rhs=hs_pair[kc][:, :w],
                        start=(start and kc == 0),
                        stop=(stop and kc == 1),
                    )

            def leaf_block(col, w, c0, h0b, dst):
                """Gates for w leaves at xT col `col`; write c0 (fp32) and h0b
                (bf16) slices, DMA fp32 h directly to hT."""
                x_t = load_x(col, w)
                g_sb = {}
                for g in (GATE_I, GATE_U, GATE_O):
                    tiles = []
                    for mc in range(2):
                        pw = MC[mc][1] - MC[mc][0]
                        ps = new_psum(mc, w)
                        xgate_mms(ps, x_t, g, mc, w)
                        t = gates.tile([pw, w], F32, tag=f"g{g}{mc}", name=f"g{g}{mc}")
                        nc.scalar.activation(out=t, in_=ps[:, :w], func=GFUNC[g],
                                             bias=bias_t[mc][:, g:g + 1])
                        tiles.append(t)
                    g_sb[g] = tiles
                for mc in range(2):
                    pw = MC[mc][1] - MC[mc][0]
                    cs = c0[mc][:, dst:dst + w]
                    nc.vector.tensor_mul(out=cs, in0=g_sb[GATE_I][mc],
                                         in1=g_sb[GATE_U][mc])
                    tt = gates.tile([pw, w], F32, tag=f"t{mc}", name=f"t{mc}")
                    nc.scalar.activation(out=tt, in_=cs, func=AF.Tanh)
                    hh = gates.tile([pw, w], F32, tag=f"h{mc}", name=f"h{mc}")
                    nc.vector.tensor_mul(out=hh, in0=g_sb[GATE_O][mc], in1=tt)
                    p0 = MC[mc][0]
                    nc.gpsimd.dma_start(out=hT[p0:MC[mc][1], col:col + w], in_=hh)
                    nc.vector.tensor_copy(out=h0b[mc][:, dst:dst + w], in_=hh)

            def internal_step(Fp, xcol, chb, cc, hb_out, cout, scol,
                              extra_h_dma=None):
                """Fp parents at xT col `xcol`; children chb (bf16, ·,4Fp) and
                cc (fp32).  Writes bf16 h into hb_out[:, scol:+Fp], fp32 c into
                cout[:, scol:+Fp], DMAs fp32 h to hT."""
                w4 = 4 * Fp
                x_t = load_x(xcol, Fp)
                # xf' = x @ W_fx  (psum) -> sbuf
                xf = []
                for mc in range(2):
                    pw = MC[mc][1] - MC[mc][0]
                    ps = new_psum(mc, Fp)
                    xgate_mms(ps, x_t, GATE_F, mc, Fp)
                    t = gates.tile([pw, Fp], F32, tag=f"xf{mc}", name=f"xf{mc}")
                    nc.vector.tensor_copy(out=t, in_=ps[:, :Fp])
                    xf.append(t)
                # f = sigmoid(chb @ W_fh + xf_bcast + b_f) over 4Fp children
                f_sb = [fwide.tile([128, w4], F32, tag="fm", name="fm"),
                        fwide.tile([22, w4], F32, tag="fr", name="fr")]
                nsl = (w4 + 511) // 512
                for ns in range(nsl):
                    a0, a1 = ns * 512, min(w4, (ns + 1) * 512)
                    cw = a1 - a0
                    pn = cw // 4
                    for mc in range(2):
                        ps = new_psum(mc, cw)
                        for kc in range(2):
                            nc.tensor.matmul(
                                out=ps[:, :cw],
                                lhsT=wh_t[kc][:, GATE_F * 150 + MC[mc][0]:
                                              GATE_F * 150 + MC[mc][1]],
                                rhs=chb[kc][:, a0:a1],
                                start=(kc == 0), stop=(kc == 1),
                            )
                        xfs = xf[mc][:, ns * 128: ns * 128 + pn]
                        xf_bc = bass.AP(tensor=xfs.tensor, offset=xfs.offset,
                                        ap=[*list(xfs.ap), [0, 4]])
                        nc.vector.tensor_add(
                            out=f_sb[mc][:, a0:a1].rearrange("p (a b) -> p a b", b=4),
                            in0=ps[:, :cw].rearrange("p (a b) -> p a b", b=4),
                            in1=xf_bc,
                        )
                for mc in range(2):
                    nc.scalar.activation(out=f_sb[mc], in_=f_sb[mc], func=AF.Sigmoid,
                                         bias=bias_t[mc][:, GATE_F:GATE_F + 1])
                # fc = group4_sum(f * cc);  hs = group4_sum(chb)
                fc, hs = [], []
                for mc in range(2):
                    pw = MC[mc][1] - MC[mc][0]
                    nc.vector.tensor_mul(out=f_sb[mc], in0=f_sb[mc],
                                         in1=cc[mc][:, :w4])
                    t = gates.tile([pw, Fp], F32, tag=f"fc{mc}", name=f"fc{mc}")
                    nc.vector.tensor_reduce(
                        out=t, in_=f_sb[mc].rearrange("p (a b) -> p a b", b=4),
                        axis=mybir.AxisListType.X, op=ALU.add)
                    fc.append(t)
                    t2 = gates.tile([pw, Fp], F32, tag=f"hsf{mc}", name=f"hsf{mc}")
                    nc.vector.tensor_reduce(
                        out=t2, in_=chb[mc][:, :w4].rearrange("p (a b) -> p a b", b=4),
                        axis=mybir.AxisListType.X, op=ALU.add)
                    t2b = gates.tile([pw, Fp], BF16, tag=f"hsb{mc}", name=f"hsb{mc}")
                    nc.vector.tensor_copy(out=t2b, in_=t2)
                    hs.append(t2b)
                # i, u, o gates
                g_sb = {}
                for g in (GATE_I, GATE_U, GATE_O):
                    tiles = []
                    for mc in range(2):
                        pw = MC[mc][1] - MC[mc][0]
                        ps = new_psum(mc, Fp)
                        xgate_mms(ps, x_t, g, mc, Fp, start=True, stop=False)
                        hgate_mms(ps, hs, g, mc, Fp, start=False, stop=True)
                        t = gates.tile([pw, Fp], F32, tag=f"g{g}{mc}", name=f"g{g}{mc}")
                        nc.scalar.activation(out=t, in_=ps[:, :Fp], func=GFUNC[g],
                                             bias=bias_t[mc][:, g:g + 1])
                        tiles.append(t)
                    g_sb[g] = tiles
                # c = i*u + fc ; h = o * tanh(c)
                for mc in range(2):
                    pw = MC[mc][1] - MC[mc][0]
                    cs = cout[mc][:, scol:scol + Fp]
                    nc.vector.tensor_mul(out=cs, in0=g_sb[GATE_I][mc],
                                         in1=g_sb[GATE_U][mc])
                    nc.vector.tensor_add(out=cs, in0=cs, in1=fc[mc])
                    tt = gates.tile([pw, Fp], F32, tag=f"t{mc}", name=f"t{mc}")
                    nc.scalar.activation(out=tt, in_=cs, func=AF.Tanh)
                    hh = gates.tile([pw, Fp], F32, tag=f"h{mc}", name=f"h{mc}")
                    nc.vector.tensor_mul(out=hh, in0=g_sb[GATE_O][mc], in1=tt)
                    p0, p1 = MC[mc]
                    nc.gpsimd.dma_start(out=hT[p0:p1, xcol:xcol + Fp], in_=hh)
                    if extra_h_dma is not None:
                        nc.gpsimd.dma_start(out=extra_h_dma[mc], in_=hh)
                    nc.vector.tensor_copy(out=hb_out[mc][:, scol:scol + Fp], in_=hh)

            # ---- persistent state: bf16 h, fp32 c, levels 1..6 ----
            st = {}
            for d in range(1, 7):
                st[d] = {
                    "h": [state.tile([128, S[d]], BF16, tag=f"h{d}m", name=f"h{d}m"),
                          state.tile([22, S[d]], BF16, tag=f"h{d}r", name=f"h{d}r")],
                    "c": [state.tile([128, S[d]], F32, tag=f"c{d}m", name=f"c{d}m"),
                          state.tile([22, S[d]], F32, tag=f"c{d}r", name=f"c{d}r")],
                }

            # ---- fused level 0 -> level 1, 4 chunks of 512 L1-parents ----
            c0 = [leafst.tile([128, 2048], F32, tag="c0m", name="c0m"),
                  leafst.tile([22, 2048], F32, tag="c0r", name="c0r")]
            h0b = [leafst.tile([128, 2048], BF16, tag="h0m", name="h0m"),
                   leafst.tile([22, 2048], BF16, tag="h0r", name="h0r")]
            for chunk in range(4):
                for s in range(4):
                    leaf_block(chunk * 2048 + s * 512, 512, c0, h0b, s * 512)
                internal_step(512, OFF[1] + chunk * 512, h0b, c0,
                              st[1]["h"], st[1]["c"], chunk * 512)

            # ---- levels 2..6 ----
            for d in range(2, 7):
                extra = None
                if d == 6:
                    extra = [hc6[0:128, 0:2], hc6[128:150, 0:2]]
                internal_step(S[d], OFF[d], st[d - 1]["h"], st[d - 1]["c"],
                              st[d]["h"], st[d]["c"], 0, extra_h_dma=extra)

            # ---- export level-6 c (fp32 state) ----
            nc.gpsimd.dma_start(out=hc6[0:128, 2:4], in_=st[6]["c"][0])
            nc.gpsimd.dma_start(out=hc6[128:150, 2:4], in_=st[6]["c"][1])
    nc.finalize()
    return nc


_NC_CACHE = None


def _get_program():
    global _NC_CACHE
    if _NC_CACHE is None:
        _NC_CACHE = _build_program()
    return _NC_CACHE


def _host_top_levels(h6, c6, embs, Ws, bs):
    """Finish levels 7 (4 nodes) and 8 (1 node) in numpy fp32."""
    (W_ix, W_fx, W_ux, W_ox, W_ih, W_fh, W_uh, W_oh) = Ws
    (b_ix, b_fx, b_ux, b_ox, b_ih, b_fh, b_uh, b_oh) = bs
    sig = lambda x: 1.0 / (1.0 + np.exp(-x, dtype=np.float32))
    h_prev, c_prev = h6, c6
    outs = []
    for d in (7, 8):
        n = SIZES[d]
        x = embs[GOFF[d]:GOFF[d] + n]
        ch = h_prev.reshape(n, K, MEM)
        cc = c_prev.reshape(n, K, MEM)
        hsum = ch.sum(axis=1)
        f = sig(np.einsum("nkm,mp->nkp", ch, W_fh) + b_fh + (x @ W_fx + b_fx)[:, None, :])
        fc = (f * cc).sum(axis=1)
        i_g = sig(x @ W_ix + b_ix + hsum @ W_ih + b_ih)
        o_g = sig(x @ W_ox + b_ox + hsum @ W_oh + b_oh)
        u = np.tanh(x @ W_ux + b_ux + hsum @ W_uh + b_uh)
        c = i_g * u + fc
        h = o_g * np.tanh(c)
        outs.append(h.astype(np.float32))
        h_prev, c_prev = h, c
    return outs


def kernel(embs, W_ix, b_ix, W_fx, b_fx, W_ux, b_ux, W_ox, b_ox,
           W_ih, b_ih, W_fh, b_fh, W_uh, b_uh, W_oh, b_oh):
    embs = np.asarray(embs, np.float32)
    Wd = {k: np.asarray(v, np.float32) for k, v in dict(
        W_ix=W_ix, b_ix=b_ix, W_fx=W_fx, b_fx=b_fx, W_ux=W_ux, b_ux=b_ux,
        W_ox=W_ox, b_ox=b_ox, W_ih=W_ih, b_ih=b_ih, W_fh=W_fh, b_fh=b_fh,
        W_uh=W_uh, b_uh=b_uh, W_oh=W_oh, b_oh=b_oh).items()}

    BF = ml_dtypes.bfloat16
    embsT = np.ascontiguousarray(embs.T).astype(BF)           # (300, N) bf16
    wx_cat = np.ascontiguousarray(np.concatenate(
        [Wd["W_ix"], Wd["W_ux"], Wd["W_ox"], Wd["W_fx"]], axis=1)).astype(BF)
    wh_cat = np.ascontiguousarray(np.concatenate(
        [Wd["W_ih"], Wd["W_uh"], Wd["W_oh"], Wd["W_fh"]], axis=1)).astype(BF)
    bias_cat = np.stack([Wd["b_ix"] + Wd["b_ih"], Wd["b_ux"] + Wd["b_uh"],
                         Wd["b_ox"] + Wd["b_oh"], Wd["b_fx"] + Wd["b_fh"]],
                        axis=1).astype(np.float32)            # (150, 4)

    in_maps = []
    for c in range(NCORES):
        blocks = [embsT[:, GOFF[d] + c * S[d]: GOFF[d] + (c + 1) * S[d]]
                  for d in range(7)]
        xT_c = np.ascontiguousarray(np.concatenate(blocks, axis=1))
        in_maps.append({"xT": xT_c, "wx": wx_cat, "wh": wh_cat, "bias": bias_cat})

    nc = _get_program()
    global LAST_IN_MAPS, LAST_EXEC_NS
    LAST_IN_MAPS = in_maps
    res = run_bass_kernel_spmd(nc, in_maps, core_ids=list(range(NCORES)))
    LAST_EXEC_NS = res.exec_time_ns

    out = np.empty((N, MEM), np.float32)
    h6_full = np.empty((16, MEM), np.float32)
    c6_full = np.empty((16, MEM), np.float32)
    for c in range(NCORES):
        hT_c = res.results[c]["hT"]                           # (150, 10922)
        for d in range(7):
            out[GOFF[d] + c * S[d]: GOFF[d] + (c + 1) * S[d]] = \
                hT_c[:, OFF[d]:OFF[d] + S[d]].T
        hc6_c = res.results[c]["hc6"]                         # (150, 4)
        h6_full[2 * c: 2 * c + 2] = hc6_c[:, 0:2].T
        c6_full[2 * c: 2 * c + 2] = hc6_c[:, 2:4].T

    Ws = (Wd["W_ix"], Wd["W_fx"], Wd["W_ux"], Wd["W_ox"],
          Wd["W_ih"], Wd["W_fh"], Wd["W_uh"], Wd["W_oh"])
    bs = (Wd["b_ix"], Wd["b_fx"], Wd["b_ux"], Wd["b_ox"],
          Wd["b_ih"], Wd["b_fh"], Wd["b_uh"], Wd["b_oh"])
    h7, h8 = _host_top_levels(h6_full, c6_full, embs, Ws, bs)
    out[GOFF[7]:GOFF[7] + 4] = h7
    out[GOFF[8]:GOFF[8] + 1] = h8
    return out


# revision 10
# speedup vs baseline: 1.0045x; 1.0045x over previous
"""Child-Sum TreeLSTM over a complete 4-ary forest — Trainium2 Bass kernel.

Layout: "transposed space" — memory dim (150) on SBUF partitions (split
128+22), nodes on the free dim.  Each of the 8 cores owns a contiguous 1/8
shard of every level 0..6; children of a core's parents at level d are
exactly the core's shard of level d-1, so levels 0..6 need no cross-core
communication.  Levels 7 (4 nodes) + 8 (1 node) are finished on the host
from each core's exported level-6 h/c (5 of 87381 nodes).

Matmul operands (x, weights, child-h) are bf16; PSUM accumulation, gates,
biases, c-state and all outputs stay fp32.

Per-core device inputs:
  xT   (300, 10922) bf16  embs^T, level blocks [L0 8192|L1 2048|...|L6 2]
  wx   (300, 600)   bf16  [W_ix | W_ux | W_ox | W_fx]
  wh   (150, 600)   bf16  [W_ih | W_uh | W_oh | W_fh]
  bias (150, 4)     fp32  combined [b_ix+b_ih, b_ux+b_uh, b_ox+b_oh, b_fx+b_fh]
Outputs (fp32):
  hT   (150, 10922)  h for the core's rows of levels 0..6 (transposed)
  hc6  (150, 4)      [h6 n0, h6 n1, c6 n0, c6 n1] for host top-levels
"""

import sys
import numpy as np
import ml_dtypes

for p in ("/opt/trn_rl_repo",):
    if p not in sys.path:
        sys.path.append(p)

import concourse.bass as bass
import concourse.bacc as bacc
import concourse.tile as tile
from concourse import mybir
from concourse.bass_utils import run_bass_kernel_spmd

F32 = mybir.dt.float32
BF16 = mybir.dt.bfloat16
LAST_EXEC_NS = None
LAST_IN_MAPS = None
AF = mybir.ActivationFunctionType
ALU = mybir.AluOpType

IN_DIM, MEM, K, D = 300, 150, 4, 9
SIZES = [K ** (D - 1 - d) for d in range(D)]          # [65536, ..., 1]
N = sum(SIZES)                                        # 87381
NCORES = 8
S = [SIZES[d] // NCORES for d in range(7)]            # [8192,2048,512,128,32,8,2]
NC_COLS = sum(S)                                      # 10922
OFF = [0]
for d in range(7):
    OFF.append(OFF[-1] + S[d])
GOFF = [0]
for d in range(D):
    GOFF.append(GOFF[-1] + SIZES[d])

KC_X = [(0, 128), (128, 256), (256, 300)]             # K chunks of IN_DIM
KC_H = [(0, 128), (128, 150)]                         # K chunks of MEM
MC = [(0, 128), (128, 150)]                           # M chunks of MEM
GATE_I, GATE_U, GATE_O, GATE_F = 0, 1, 2, 3
GFUNC = {GATE_I: AF.Sigmoid, GATE_U: AF.Tanh, GATE_O: AF.Sigmoid}


def _build_program():
    nc = bacc.Bacc()
    xT = nc.declare_dram_parameter("xT", [IN_DIM, NC_COLS], BF16, isOutput=False)
    wx = nc.declare_dram_parameter("wx", [IN_DIM, 600], BF16, isOutput=False)
    wh = nc.declare_dram_parameter("wh", [MEM, 600], BF16, isOutput=False)
    bias = nc.declare_dram_parameter("bias", [MEM, 4], F32, isOutput=False)
    hT = nc.declare_dram_parameter("hT", [MEM, NC_COLS], F32, isOutput=True)
    hc6 = nc.declare_dram_parameter("hc6", [MEM, 4], F32, isOutput=True)

    with tile.TileContext(nc) as tc:
        with (
            tc.tile_pool(name="consts", bufs=1) as consts,
            tc.tile_pool(name="xs", bufs=3) as xs,
            tc.tile_pool(name="gates", bufs=2) as gates,
            tc.tile_pool(name="fwide", bufs=1) as fwide,
            tc.tile_pool(name="leafst", bufs=1) as leafst,
            tc.tile_pool(name="state", bufs=1) as state,
            tc.tile_pool(name="pm", bufs=4, space="PSUM") as pmpool,
            tc.tile_pool(name="pr", bufs=4, space="PSUM") as prpool,
        ):
            # ---- load weights / biases once ----
            wx_t, wh_t = [], []
            for i, (a, b) in enumerate(KC_X):
                t = consts.tile([b - a, 600], BF16, tag=f"wx{i}", name=f"wx{i}")
                nc.gpsimd.dma_start(out=t, in_=wx[a:b, :])
                wx_t.append(t)
            for i, (a, b) in enumerate(KC_H):
                t = consts.tile([b - a, 600], BF16, tag=f"wh{i}", name=f"wh{i}")
                nc.gpsimd.dma_start(out=t, in_=wh[a:b, :])
                wh_t.append(t)
            bias_t = []
            for i, (a, b) in enumerate(MC):
                t = consts.tile([b - a, 4], F32, tag=f"b{i}", name=f"b{i}")
                nc.gpsimd.dma_start(out=t, in_=bias[a:b, :])
                bias_t.append(t)
            # ACT-engine warmup touch of each bias tile: absorbs the bias-DMA
            # wait into a tiny 1-wait instruction so later psum-evac ACTs only
            # wait on PE (walrus allows very few sync commands per instr).
            for i in range(2):
                wu = consts.tile([MC[i][1] - MC[i][0], 1], F32,
                                 tag=f"wu{i}", name=f"wu{i}")
                nc.scalar.copy(out=wu, in_=bias_t[i][:, 0:1])

            def load_x(col, w):
                ts = []
                for i, (a, b) in enumerate(KC_X):
                    t = xs.tile([b - a, w], BF16, tag=f"x{i}", name=f"x{i}")
                    nc.gpsimd.dma_start(out=t, in_=xT[a:b, col:col + w])
                    ts.append(t)
                return ts

            def new_psum(mc, w):
                if mc == 0:
                    return pmpool.tile([128, w], F32, tag="pm", name="pm")
                return prpool.tile([22, w], F32, tag="pr", name="pr")

            def xgate_mms(ps, x_t, g, mc, w, start=True, stop=True):
                m0, m1 = MC[mc]
                for kc in range(3):
                    nc.tensor.matmul(
                        out=ps[:, :w],
                        lhsT=wx_t[kc][:, g * 150 + m0: g * 150 + m1],
                        rhs=x_t[kc][:, :w],
                        start=(start and kc == 0),
                        stop=(stop and kc == 2),
                    )

            def hgate_mms(ps, hs_pair, g, mc, w, start, stop):
                m0, m1 = MC[mc]
                for kc in range(2):
                    nc.tensor.matmul(
                        out=ps[:, :w],
                        lhsT=wh_t[kc][:, g * 150 + m0: g * 150 + m1],
                        rhs=hs_pair[kc][:, :w],
                        start=(start and kc == 0),
                        stop=(stop and kc == 1),
                    )

            def leaf_block(col, w, c0, h0b, dst):
                """Gates for w leaves at xT col `col`; write c0 (fp32) and h0b
                (bf16) slices, DMA fp32 h directly to hT."""
                x_t = load_x(col, w)
                g_sb = {}
                for g in (GATE_I, GATE_U, GATE_O):
                    tiles = []
                    for mc in range(2):
                        pw = MC[mc][1] - MC[mc][0]
                        ps = new_psum(mc, w)
                        xgate_mms(ps, x_t, g, mc, w)
                        t = gates.tile([pw, w], F32, tag=f"g{g}{mc}", name=f"g{g}{mc}")
                        nc.scalar.activation(out=t, in_=ps[:, :w], func=GFUNC[g],
                                             bias=bias_t[mc][:, g:g + 1])
                        tiles.append(t)
                    g_sb[g] = tiles
                for mc in range(2):
                    pw = MC[mc][1] - MC[mc][0]
                    cs = c0[mc][:, dst:dst + w]
                    nc.vector.tensor_mul(out=cs, in0=g_sb[GATE_I][mc],
                                         in1=g_sb[GATE_U][mc])
                    tt = gates.tile([pw, w], F32, tag=f"t{mc}", name=f"t{mc}")
                    nc.scalar.activation(out=tt, in_=cs, func=AF.Tanh)
                    hh = gates.tile([pw, w], F32, tag=f"h{mc}", name=f"h{mc}")
                    nc.vector.tensor_mul(out=hh, in0=g_sb[GATE_O][mc], in1=tt)
                    p0 = MC[mc][0]
                    nc.gpsimd.dma_start(out=hT[p0:MC[mc][1], col:col + w], in_=hh)
                    nc.gpsimd.tensor_copy(out=h0b[mc][:, dst:dst + w], in_=hh)

            def internal_step(Fp, xcol, chb, cc, hb_out, cout, scol,
                              extra_h_dma=None):
                """Fp parents at xT col `xcol`; children chb (bf16, ·,4Fp) and
                cc (fp32).  Writes bf16 h into hb_out[:, scol:+Fp], fp32 c into
                cout[:, scol:+Fp], DMAs fp32 h to hT."""
                w4 = 4 * Fp
                x_t = load_x(xcol, Fp)
                # f = sigmoid(chb @ W_fh + (x @ W_fx)_bcast + b_f): the x-term
                # is accumulated into the same psum by streaming each x column
                # 4x via a 0-stride AP axis (one value per child).
                f_sb = [fwide.tile([128, w4], F32, tag="fm", name="fm"),
                        fwide.tile([22, w4], F32, tag="fr", name="fr")]
                nsl = (w4 + 511) // 512
                for ns in range(nsl):
                    a0, a1 = ns * 512, min(w4, (ns + 1) * 512)
                    cw = a1 - a0
                    pn = cw // 4
                    for mc in range(2):
                        m0, m1 = MC[mc]
                        ps = new_psum(mc, cw)
                        for kc in range(2):
                            nc.tensor.matmul(
                                out=ps[:, :cw],
                                lhsT=wh_t[kc][:, GATE_F * 150 + m0:
                                              GATE_F * 150 + m1],
                                rhs=chb[kc][:, a0:a1],
                                start=(kc == 0), stop=False,
                            )
                        for kc in range(3):
                            xs_ = x_t[kc][:, ns * 128: ns * 128 + pn]
                            x_rep = bass.AP(tensor=xs_.tensor, offset=xs_.offset,
                                            ap=[*list(xs_.ap), [0, 4]])
                            nc.tensor.matmul(
                                out=ps[:, :cw].rearrange("p (a b) -> p a b", b=4),
                                lhsT=wx_t[kc][:, GATE_F * 150 + m0:
                                              GATE_F * 150 + m1],
                                rhs=x_rep,
                                start=False, stop=(kc == 2),
                            )
                        nc.scalar.activation(
                            out=f_sb[mc][:, a0:a1], in_=ps[:, :cw],
                            func=AF.Sigmoid,
                            bias=bias_t[mc][:, GATE_F:GATE_F + 1])
                # fc = group4_sum(f * cc);  hs = group4_sum(chb)
                fc, hs = [], []
                for mc in range(2):
                    pw = MC[mc][1] - MC[mc][0]
                    nc.vector.tensor_mul(out=f_sb[mc], in0=f_sb[mc],
                                         in1=cc[mc][:, :w4])
                    t = gates.tile([pw, Fp], F32, tag=f"fc{mc}", name=f"fc{mc}")
                    nc.vector.tensor_reduce(
                        out=t, in_=f_sb[mc].rearrange("p (a b) -> p a b", b=4),
                        axis=mybir.AxisListType.X, op=ALU.add)
                    fc.append(t)
                    t2 = gates.tile([pw, Fp], F32, tag=f"hsf{mc}", name=f"hsf{mc}")
                    nc.vector.tensor_reduce(
                        out=t2, in_=chb[mc][:, :w4].rearrange("p (a b) -> p a b", b=4),
                        axis=mybir.AxisListType.X, op=ALU.add)
                    t2b = gates.tile([pw, Fp], BF16, tag=f"hsb{mc}", name=f"hsb{mc}")
                    nc.vector.tensor_copy(out=t2b, in_=t2)
                    hs.append(t2b)
                # i, u, o gates
                g_sb = {}
                for g in (GATE_I, GATE_U, GATE_O):
                    tiles = []
                    for mc in range(2):
                        pw = MC[mc][1] - MC[mc][0]
                        ps = new_psum(mc, Fp)
                        xgate_mms(ps, x_t, g, mc, Fp, start=True, stop=False)
                        hgate_mms(ps, hs, g, mc, Fp, start=False, stop=True)
                        t = gates.tile([pw, Fp], F32, tag=f"g{g}{mc}", name=f"g{g}{mc}")
                        nc.scalar.activation(out=t, in_=ps[:, :Fp], func=GFUNC[g],
                                             bias=bias_t[mc][:, g:g + 1])
                        tiles.append(t)
                    g_sb[g] = tiles
                # c = i*u + fc ; h = o * tanh(c)
                for mc in range(2):
                    pw = MC[mc][1] - MC[mc][0]
                    cs = cout[mc][:, scol:scol + Fp]
                    nc.vector.tensor_mul(out=cs, in0=g_sb[GATE_I][mc],
                                         in1=g_sb[GATE_U][mc])
                    nc.vector.tensor_add(out=cs, in0=cs, in1=fc[mc])
                    tt = gates.tile([pw, Fp], F32, tag=f"t{mc}", name=f"t{mc}")
                    nc.scalar.activation(out=tt, in_=cs, func=AF.Tanh)
                    hh = gates.tile([pw, Fp], F32, tag=f"h{mc}", name=f"h{mc}")
                    nc.vector.tensor_mul(out=hh, in0=g_sb[GATE_O][mc], in1=tt)
                    p0, p1 = MC[mc]
                    nc.gpsimd.dma_start(out=hT[p0:p1, xcol:xcol + Fp], in_=hh)
                    if extra_h_dma is not None:
                        nc.gpsimd.dma_start(out=extra_h_dma[mc], in_=hh)
                    nc.gpsimd.tensor_copy(out=hb_out[mc][:, scol:scol + Fp], in_=hh)

            # ---- persistent state: bf16 h, fp32 c, levels 1..6 ----
            st = {}
            for d in range(1, 7):
                st[d] = {
                    "h": [state.tile([128, S[d]], BF16, tag=f"h{d}m", name=f"h{d}m"),
                          state.tile([22, S[d]], BF16, tag=f"h{d}r", name=f"h{d}r")],
                    "c": [state.tile([128, S[d]], F32, tag=f"c{d}m", name=f"c{d}m"),
                          state.tile([22, S[d]], F32, tag=f"c{d}r", name=f"c{d}r")],
                }

            # ---- fused level 0 -> level 1, 4 chunks of 512 L1-parents ----
            c0 = [leafst.tile([128, 2048], F32, tag="c0m", name="c0m"),
                  leafst.tile([22, 2048], F32, tag="c0r", name="c0r")]
            h0b = [leafst.tile([128, 2048], BF16, tag="h0m", name="h0m"),
                   leafst.tile([22, 2048], BF16, tag="h0r", name="h0r")]
            for chunk in range(4):
                for s in range(4):
                    leaf_block(chunk * 2048 + s * 512, 512, c0, h0b, s * 512)
                internal_step(512, OFF[1] + chunk * 512, h0b, c0,
                              st[1]["h"], st[1]["c"], chunk * 512)

            # ---- levels 2..6 ----
            for d in range(2, 7):
                extra = None
                if d == 6:
                    extra = [hc6[0:128, 0:2], hc6[128:150, 0:2]]
                internal_step(S[d], OFF[d], st[d - 1]["h"], st[d - 1]["c"],
                              st[d]["h"], st[d]["c"], 0, extra_h_dma=extra)

            # ---- export level-6 c (fp32 state) ----
            nc.gpsimd.dma_start(out=hc6[0:128, 2:4], in_=st[6]["c"][0])
            nc.gpsimd.dma_start(out=hc6[128:150, 2:4], in_=st[6]["c"][1])
    nc.finalize()
    return nc


_NC_CACHE = None


def _get_program():
    global _NC_CACHE
    if _NC_CACHE is None:
        _NC_CACHE = _build_program()
    return _NC_CACHE


def _host_top_levels(h6, c6, embs, Ws, bs):
    """Finish levels 7 (4 nodes) and 8 (1 node) in numpy fp32."""
    (W_ix, W_fx, W_ux, W_ox, W_ih, W_fh, W_uh, W_oh) = Ws
    (b_ix, b_fx, b_ux, b_ox, b_ih, b_fh, b_uh, b_oh) = bs
    sig = lambda x: 1.0 / (1.0 + np.exp(-x, dtype=np.float32))
    h_prev, c_prev = h6, c6
    outs = []
    for d in (7, 8):
        n = SIZES[d]
        x = embs[GOFF[d]:GOFF[d] + n]
        ch = h_prev.reshape(n, K, MEM)
        cc = c_prev.reshape(n, K, MEM)
        hsum = ch.sum(axis=1)
        f = sig(np.einsum("nkm,mp->nkp", ch, W_fh) + b_fh + (x @ W_fx + b_fx)[:, None, :])
        fc = (f * cc).sum(axis=1)
        i_g = sig(x @ W_ix + b_ix + hsum @ W_ih + b_ih)
        o_g = sig(x @ W_ox + b_ox + hsum @ W_oh + b_oh)
        u = np.tanh(x @ W_ux + b_ux + hsum @ W_uh + b_uh)
        c = i_g * u + fc
        h = o_g * np.tanh(c)
        outs.append(h.astype(np.float32))
        h_prev, c_prev = h, c
    return outs


def kernel(embs, W_ix, b_ix, W_fx, b_fx, W_ux, b_ux, W_ox, b_ox,
           W_ih, b_ih, W_fh, b_fh, W_uh, b_uh, W_oh, b_oh):
    embs = np.asarray(embs, np.float32)
    Wd = {k: np.asarray(v, np.float32) for k, v in dict(
        W_ix=W_ix, b_ix=b_ix, W_fx=W_fx, b_fx=b_fx, W_ux=W_ux, b_ux=b_ux,
        W_ox=W_ox, b_ox=b_ox, W_ih=W_ih, b_ih=b_ih, W_fh=W_fh, b_fh=b_fh,
        W_uh=W_uh, b_uh=b_uh, W_oh=W_oh, b_oh=b_oh).items()}

    BF = ml_dtypes.bfloat16
    embsT = np.ascontiguousarray(embs.T).astype(BF)           # (300, N) bf16
    wx_cat = np.ascontiguousarray(np.concatenate(
        [Wd["W_ix"], Wd["W_ux"], Wd["W_ox"], Wd["W_fx"]], axis=1)).astype(BF)
    wh_cat = np.ascontiguousarray(np.concatenate(
        [Wd["W_ih"], Wd["W_uh"], Wd["W_oh"], Wd["W_fh"]], axis=1)).astype(BF)
    bias_cat = np.stack([Wd["b_ix"] + Wd["b_ih"], Wd["b_ux"] + Wd["b_uh"],
                         Wd["b_ox"] + Wd["b_oh"], Wd["b_fx"] + Wd["b_fh"]],
                        axis=1).astype(np.float32)            # (150, 4)

    in_maps = []
    for c in range(NCORES):
        blocks = [embsT[:, GOFF[d] + c * S[d]: GOFF[d] + (c + 1) * S[d]]
                  for d in range(7)]
        xT_c = np.ascontiguousarray(np.concatenate(blocks, axis=1))
        in_maps.append({"xT": xT_c, "wx": wx_cat, "wh": wh_cat, "bias": bias_cat})

    nc = _get_program()
    global LAST_IN_MAPS, LAST_EXEC_NS
    LAST_IN_MAPS = in_maps
    res = run_bass_kernel_spmd(nc, in_maps, core_ids=list(range(NCORES)))
    LAST_EXEC_NS = res.exec_time_ns

    out = np.empty((N, MEM), np.float32)
    h6_full = np.empty((16, MEM), np.float32)
    c6_full = np.empty((16, MEM), np.float32)
    for c in range(NCORES):
        hT_c = res.results[c]["hT"]                           # (150, 10922)
        for d in range(7):
            out[GOFF[d] + c * S[d]: GOFF[d] + (c + 1) * S[d]] = \
                hT_c[:, OFF[d]:OFF[d] + S[d]].T
        hc6_c = res.results[c]["hc6"]                         # (150, 4)
        h6_full[2 * c: 2 * c + 2] = hc6_c[:, 0:2].T
        c6_full[2 * c: 2 * c + 2] = hc6_c[:, 2:4].T

    Ws = (Wd["W_ix"], Wd["W_fx"], Wd["W_ux"], Wd["W_ox"],
          Wd["W_ih"], Wd["W_fh"], Wd["W_uh"], Wd["W_oh"])
    bs = (Wd["b_ix"], Wd["b_fx"], Wd["b_ux"], Wd["b_ox"],
          Wd["b_ih"], Wd["b_fh"], Wd["b_uh"], Wd["b_oh"])
    h7, h8 = _host_top_levels(h6_full, c6_full, embs, Ws, bs)
    out[GOFF[7]:GOFF[7] + 4] = h7
    out[GOFF[8]:GOFF[8] + 1] = h8
    return out
